# revision 3
# baseline (speedup 1.0000x reference)
"""GQA attention (B=1, S=2048, HID=4096, 32 q-heads / 8 kv-heads, HD=128) on 8
Trainium2 NeuronCores — v6.

v6 changes vs v5 (driven by the v5 trace):
  - All exp-sum accumulation on DVE with a single accumulator (v5 put
    odd tiles on gpsimd, whose slow tensor ops delayed the causal-mask
    selects that gate AV -> recurring 5-7us PE stalls).
  - Broadcast of 1/denom back to bf16 (f32 partition_broadcast was
    1.8us per head on gpsimd).

v5 changes vs v4:
  - DMA queues reassigned so nothing time-critical queues behind a
    stalled write: sync = hidden loads only; scalar = weight loads +
    ao/out writes; gpsimd = o_proj staging loads.
  - Softmax denominator off the PE: exp tiles accumulated into bf16
    running sums, then a 1-row PE matmul per head reduces them (was
    160 full-width PE passes).
  - reciprocal_approx_fast (f32) instead of vector.reciprocal (5x).

v4 changes vs v3 (driven by the v3 trace):
  - All weight/hidden tensors host-pre-tiled to the exact SBUF layout so
    every load DMA is contiguous per partition (v3's strided rearrange
    loads cost 3-10us of descriptor-build EACH on the single sync queue;
    first matmul started at 35us).
  - DMAs spread across the three DGE-capable queues: sync (hidden, ao,
    out), scalar (weight loads, done before exp traffic starts), gpsimd
    (attention-output staging reads for o_proj).
  - Weight loads issued before const loads; first matmul needs only wk +
    first hidden sub-tile.
  - Scores prefetch depth 2 (pS bufs=3) so exp latency never stalls AV.
  - Final chunk's AllGather split into two half-collectives (heads 0-1
    fired early) so the ~30us collective latency hides behind the last
    attention + o_proj of chunk 2.
"""
import math
from contextlib import ExitStack

import numpy as np
import ml_dtypes

import concourse.bass as bass
import concourse.tile as tile
from concourse import bacc, mybir

f32 = mybir.dt.float32
bf16 = mybir.dt.bfloat16

S, HID, NH, NKV, HD = 2048, 4096, 32, 8, 128
N_CORES = 8
HPC = NH // N_CORES           # 4 q heads per core
DQ = HPC * HD                 # 512 q/o columns per core
NCHUNK = S // 512             # 4 sq chunks
NT = HID // 128               # 32 hid tiles
SCALE = 1.0 / math.sqrt(HD)
ROPE_THETA = 10000.0

# even/odd de-interleave permutation of the head dim (applied host-side to
# wq/wk columns and to cos/sin rows); makes rotate_interleaved a 64-partition
# half-swap with sign, which runs on DVE instead of the PE.
PERM = np.concatenate([np.arange(0, HD, 2), np.arange(1, HD, 2)])


# ---------------------------------------------------------------- constants
def host_constants():
    inv = (1.0 / (ROPE_THETA ** (np.arange(0, HD, 2, dtype=np.float32) / HD))
           ).astype(np.float32)
    t = np.arange(S, dtype=np.float32)
    freqs = np.outer(t, inv).astype(np.float32)          # [S, 64]
    emb = np.concatenate([freqs, freqs], axis=1)         # [S, 128]
    cos = np.cos(emb).T                                  # [HD, S]
    sin = np.sin(emb).T
    cosP = np.ascontiguousarray(cos[PERM]).astype(ml_dtypes.bfloat16)
    sinS = sin[PERM].copy()
    sinS[:64] *= -1.0        # top half of roped out = q*cos - q_hi*sin
    sinS = np.ascontiguousarray(sinS).astype(ml_dtypes.bfloat16)
    ident = np.eye(128, dtype=ml_dtypes.bfloat16)
    ones = np.ones((128, 8), dtype=ml_dtypes.bfloat16)
    return {"cosP": cosP, "sinS": sinS, "ident": ident, "ones": ones}


def _tile_w(w):
    """[HID, D] -> [128, NT, D] with w_t[p, t, d] = w[128t+p, d]."""
    D = w.shape[1]
    return np.ascontiguousarray(w.reshape(NT, 128, D).transpose(1, 0, 2))


# ---------------------------------------------------------------- bass build
def build_nc(n_cores=N_CORES, with_collective=True):
    nc = bacc.Bacc("TRN2", target_bir_lowering=False, debug=False,
                   num_devices=n_cores)
    # hidden pre-tiled: hidH[p, j, t, s] = hidden[512j+s, 128t+p]
    hid_d = nc.dram_tensor("hidH", [128, NCHUNK, NT, 512], bf16,
                           kind="ExternalInput").ap()
    # weights pre-tiled: [128, (m,) NT, d]
    wq_d = nc.dram_tensor("wqT", [128, HPC, NT, 128], bf16,
                          kind="ExternalInput").ap()
    wk_d = nc.dram_tensor("wkT", [128, NT, 128], bf16,
                          kind="ExternalInput").ap()
    wv_d = nc.dram_tensor("wvT", [128, NT, 128], bf16,
                          kind="ExternalInput").ap()
    wo_d = nc.dram_tensor("woT", [128, NT, 512], bf16,
                          kind="ExternalInput").ap()
    cos_d = nc.dram_tensor("cosP", [HD, S], bf16, kind="ExternalInput").ap()
    sin_d = nc.dram_tensor("sinS", [HD, S], bf16, kind="ExternalInput").ap()
    ident_d = nc.dram_tensor("ident", [128, 128], bf16,
                             kind="ExternalInput").ap()
    ones_d = nc.dram_tensor("ones", [128, 8], bf16, kind="ExternalInput").ap()
    out_d = nc.dram_tensor("out", [S, DQ], f32, kind="ExternalOutput").ap()

    # per-chunk collective buffers; chunk 3 split into two half-gathers
    cc_in = [nc.dram_tensor(f"cc_in{j}", [DQ, 512], bf16, kind="Internal").ap()
             for j in range(NCHUNK)]
    space = "Shared" if with_collective else None
    kind = "Internal"
    cc_out = [nc.dram_tensor(f"cc_out{j}", [HID, 512], bf16, kind=kind,
                             addr_space=space).ap()
              for j in range(NCHUNK - 1)]
    cc_out3 = [nc.dram_tensor(f"cc_out3{half}", [HID // 2, 512], bf16,
                              kind=kind, addr_space=space).ap()
               for half in ("a", "b")]

    Exp = mybir.ActivationFunctionType.Exp

    with tile.TileContext(nc) as tc, ExitStack() as top:
        constp = top.enter_context(tc.tile_pool(name="const", bufs=1))

        with ExitStack() as ks:
            persist = ks.enter_context(tc.tile_pool(name="persist", bufs=1))
            w_sb = [persist.tile([128, NT, 128], bf16, tag=f"w{m}",
                                 name=f"w{m}") for m in range(HPC + 2)]
            wo_sb = persist.tile([128, NT, 512], bf16, tag="wo")
            kT = persist.tile([128, S], bf16, tag="kT")
            v_sb = persist.tile([128, S], bf16, tag="v_sb")

            # weight loads first (scalar queue): wk, wv needed immediately
            nc.scalar.dma_start(w_sb[0][:], wk_d[:])
            nc.scalar.dma_start(w_sb[1][:], wv_d[:])
            for m in range(HPC):
                nc.scalar.dma_start(w_sb[2 + m][:], wq_d[:, m])

            ident = constp.tile([128, 128], bf16, tag="ident")
            nc.scalar.dma_start(ident[:], ident_d[:])
            ones = constp.tile([128, 8], bf16, tag="ones")
            nc.scalar.dma_start(ones[:], ones_d[:])
            cos_sb = constp.tile([128, S], bf16, tag="cos")
            nc.scalar.dma_start(cos_sb[:], cos_d[:])
            sin_sb = constp.tile([128, S], bf16, tag="sin")
            nc.scalar.dma_start(sin_sb[:], sin_d[:])

            hTap = ks.enter_context(tc.tile_pool(name="hTa", bufs=2))
            hTbp = ks.enter_context(tc.tile_pool(name="hTb", bufs=1))
            qTp = ks.enter_context(tc.tile_pool(name="qT", bufs=2))
            tmpp = ks.enter_context(tc.tile_pool(name="tmp", bufs=2))
            rawp = ks.enter_context(tc.tile_pool(name="raw", bufs=1))
            Ep = ks.enter_context(tc.tile_pool(name="E", bufs=4))
            smp = ks.enter_context(tc.tile_pool(name="sm", bufs=2))
            rcp = ks.enter_context(tc.tile_pool(name="rc", bufs=1))
            aop = ks.enter_context(tc.tile_pool(name="ao", bufs=2))
            aTp = ks.enter_context(tc.tile_pool(name="aT", bufs=1))
            osbp = ks.enter_context(tc.tile_pool(name="osb", bufs=1))
            # PSUM: acc(2) + vtr(1) + pS(3) + pO(1) + pD(1) = 8 banks
            paccp = ks.enter_context(
                tc.tile_pool(name="pacc", bufs=2, space="PSUM"))
            pvtrp = ks.enter_context(
                tc.tile_pool(name="pvtr", bufs=1, space="PSUM"))
            pSp = ks.enter_context(
                tc.tile_pool(name="pS", bufs=3, space="PSUM"))
            pOp = ks.enter_context(
                tc.tile_pool(name="pO", bufs=1, space="PSUM"))
            pDp = ks.enter_context(
                tc.tile_pool(name="pD", bufs=1, space="PSUM"))

            def rope_dve(acc, cos_c, sinS_c, dest):
                """dest = acc*cos + halfswap(acc)*sin (all [128, 512])."""
                t1 = tmpp.tile([128, 512], bf16, tag="t1")
                nc.vector.tensor_mul(t1[:], acc[:], cos_c)
                t2 = tmpp.tile([128, 512], bf16, tag="t2")
                nc.vector.tensor_mul(t2[0:64, :], acc[64:128, :], sinS_c[0:64])
                nc.vector.tensor_mul(t2[64:128, :], acc[0:64, :],
                                     sinS_c[64:128])
                nc.vector.tensor_add(dest, t1[:], t2[:])

            def phase_o(jo):
                """o_proj for sq chunk jo (reads the gathered attention
                outputs; for jo=3 they arrive as two head-half tensors)."""
                aTa = aTp.tile([128, NT // 2, 512], bf16, tag="aTa")
                aTb = aTp.tile([128, NT // 2, 512], bf16, tag="aTb")
                if jo < NCHUNK - 1:
                    a3 = cc_out[jo].rearrange("(t p) s -> p t s", p=128)
                    for g in range(2):
                        nc.gpsimd.dma_start(
                            aTa[:, 8 * g:8 * (g + 1), :],
                            a3[:, 8 * g:8 * (g + 1), :])
                    for g in range(2):
                        nc.gpsimd.dma_start(
                            aTb[:, 8 * g:8 * (g + 1), :],
                            a3[:, 16 + 8 * g:16 + 8 * (g + 1), :])
                    # accumulation order: t = 0..31; tile t lives in
                    # aTa[t] for t<16 else aTb[t-16]
                    order = [(t, (aTa, t) if t < 16 else (aTb, t - 16))
                             for t in range(NT)]
                else:
                    a3a = cc_out3[0].rearrange("(t p) s -> p t s", p=128)
                    a3b = cc_out3[1].rearrange("(t p) s -> p t s", p=128)
                    for g in range(2):
                        nc.gpsimd.dma_start(
                            aTa[:, 8 * g:8 * (g + 1), :],
                            a3a[:, 8 * g:8 * (g + 1), :])
                    for g in range(2):
                        nc.gpsimd.dma_start(
                            aTb[:, 8 * g:8 * (g + 1), :],
                            a3b[:, 8 * g:8 * (g + 1), :])
                    # half a holds heads 0-1 of each core: global hid tile
                    # t = 4c+h -> (aTa, 2c+h) for h<2, (aTb, 2c+h-2) else
                    order = []
                    for c in range(8):
                        for h in range(2):
                            order.append((4 * c + h, (aTa, 2 * c + h)))
                    for c in range(8):
                        for h in range(2, 4):
                            order.append((4 * c + h, (aTb, 2 * c + h - 2)))
                for b in range(4):
                    acc = paccp.tile([128, 512], f32, tag="acc")
                    for n, (t, (src, idx)) in enumerate(order):
                        nc.tensor.matmul(
                            acc[:], src[:, idx, 128 * b:128 * (b + 1)],
                            wo_sb[:, t, :],
                            start=(n == 0), stop=(n == NT - 1))
                    o_sb = osbp.tile([128, 512], f32, tag="osb")
                    nc.scalar.copy(o_sb[:], acc[:])
                    sqt = 4 * jo + b
                    nc.scalar.dma_start(out_d[128 * sqt:128 * (sqt + 1), :],
                                        o_sb[:])

            for j in range(NCHUNK):
                # ---------------- phase P for chunk j: hT DMA + projections
                hTa = hTap.tile([128, NT // 2, 512], bf16, tag="hTa")
                hTb = hTbp.tile([128, NT // 2, 512], bf16, tag="hTb")
                for g in range(2):
                    nc.sync.dma_start(hTa[:, 8 * g:8 * (g + 1), :],
                                      hid_d[:, j, 8 * g:8 * (g + 1), :])
                for g in range(2):
                    nc.sync.dma_start(hTb[:, 8 * g:8 * (g + 1), :],
                                      hid_d[:, j, 16 + 8 * g:24 + 8 * g, :])

                def hT(t):
                    return hTa[:, t, :] if t < 16 else hTb[:, t - 16, :]

                cos_c = cos_sb[:, 512 * j:512 * (j + 1)]
                sin_c = sin_sb[:, 512 * j:512 * (j + 1)]
                if j == 1:   # prefetch wo for phase O during chunk 1
                    for g in range(4):
                        nc.scalar.dma_start(wo_sb[:, 8 * g:8 * (g + 1), :],
                                            wo_d[:, 8 * g:8 * (g + 1), :])

                qT = qTp.tile([128, HPC, 512], bf16, tag="qT")
                # m order: k, v, q0..q3 so attention can start earliest
                accs = {}
                for m in range(HPC + 2):
                    acc = paccp.tile([128, 512], f32, tag="acc")
                    accs[m] = acc
                    for t in range(NT):
                        nc.tensor.matmul(
                            acc[:], w_sb[m][:, t, :], hT(t),
                            start=(t == 0), stop=(t == NT - 1))
                    if m == 0:
                        rope_dve(acc, cos_c, sin_c,
                                 kT[:, 512 * j:512 * (j + 1)])
                    elif m == 1:
                        # v: drain now (ACT), transpose staggered after q0
                        raw = rawp.tile([128, 512], bf16, tag="raw")
                        nc.scalar.copy(raw[:], acc[:])
                        accs["raw_v"] = raw
                    else:
                        rope_dve(acc, cos_c, sin_c, qT[:, m - 2, :])
                        if m == 2:
                            # v transpose (PE) staggered behind q0's matmuls
                            raw = accs["raw_v"]
                            ps = pvtrp.tile([128, 512], bf16, tag="vtr")
                            for tt in range(4):
                                nc.tensor.matmul(
                                    ps[:, 128 * tt:128 * (tt + 1)],
                                    raw[:, 128 * tt:128 * (tt + 1)],
                                    ident[:], is_transpose=True,
                                    start=(tt == 0), stop=(tt == 3))
                            nc.vector.tensor_copy(
                                v_sb[:, 512 * j:512 * (j + 1)], ps[:])

                # ---------------- phase A for chunk j (all heads)
                for h in range(HPC):
                    ni = 4 * j + 4
                    acc_o = pOp.tile([128, 512], f32, tag="pO")
                    acc_d = pDp.tile([1, 512], f32, tag="pD")

                    def _delta(i):
                        return max(0, 128 * i - 512 * j)

                    def scores_mm(i):
                        d = _delta(i)
                        ps = pSp.tile([128, 512], f32, tag="pS")
                        nc.tensor.matmul(ps[:, d:],
                                         kT[:, 128 * i:128 * (i + 1)],
                                         qT[:, h, d:], start=True, stop=True)
                        return ps

                    pss = [scores_mm(0)]
                    if ni > 1:
                        pss.append(scores_mm(1))
                    # exp-sum accumulator on DVE (replaces a full-width
                    # PE pass per sk-tile); tile 0 is always full-width
                    dsA = smp.tile([128, 512], bf16, tag="dsA")
                    for i in range(ni):
                        d0 = _delta(i)
                        w = 512 - d0
                        E = Ep.tile([128, 512], bf16, tag="E")
                        nc.scalar.activation(E[:, d0:], pss[i][:, d0:], Exp,
                                             scale=SCALE)
                        if i >= 4 * j:   # diagonal-crossing tile: mask
                            nc.gpsimd.affine_select(
                                E[:, d0:], E[:, d0:], pattern=[[1, w]],
                                compare_op=mybir.AluOpType.is_ge,
                                fill=0.0, base=0,
                                channel_multiplier=-1)
                        if i + 2 < ni:
                            pss.append(scores_mm(i + 2))
                        nc.tensor.matmul(acc_o[:, d0:],
                                         v_sb[:, 128 * i:128 * (i + 1)],
                                         E[:, d0:], start=(i == 0),
                                         stop=(i == ni - 1))
                        with nc.allow_low_precision(reason="softmax denom"):
                            if i == 0:
                                nc.vector.tensor_copy(dsA[:], E[:])
                            else:
                                nc.vector.tensor_add(dsA[:, d0:], dsA[:, d0:],
                                                     E[:, d0:])
                    nc.tensor.matmul(acc_d[:], ones[:, 0:1], dsA[:],
                                     start=True, stop=True)
                    recip = rcp.tile([1, 512], f32, tag="recip")
                    nc.vector.reciprocal_approx_fast(recip[:], acc_d[:])
                    recip_bf = rcp.tile([1, 512], bf16, tag="recipb")
                    with nc.allow_low_precision(reason="softmax denom"):
                        nc.vector.tensor_copy(recip_bf[:], recip[:])
                    bc = smp.tile([128, 512], bf16, tag="bc")
                    nc.gpsimd.partition_broadcast(bc[:], recip_bf[:])
                    ao = aop.tile([128, 512], bf16, tag="ao")
                    nc.vector.tensor_mul(ao[:], acc_o[:], bc[:])
                    nc.scalar.dma_start(
                        cc_in[j][128 * h:128 * (h + 1), :], ao[:])

                    # last chunk: fire the first half-gather once heads 0-1
                    # are out, so its latency hides behind heads 2-3
                    if with_collective and j == NCHUNK - 1 and h == 1:
                        nc.gpsimd.collective_compute(
                            "AllGather", mybir.AluOpType.bypass,
                            replica_groups=[list(range(n_cores))],
                            ins=[cc_in[j][0:256, :].opt()],
                            outs=[cc_out3[0][:].opt()])

                # ---------------- AllGather for chunk j
                if with_collective:
                    if j < NCHUNK - 1:
                        nc.gpsimd.collective_compute(
                            "AllGather", mybir.AluOpType.bypass,
                            replica_groups=[list(range(n_cores))],
                            ins=[cc_in[j][:].opt()], outs=[cc_out[j][:].opt()])
                    else:
                        nc.gpsimd.collective_compute(
                            "AllGather", mybir.AluOpType.bypass,
                            replica_groups=[list(range(n_cores))],
                            ins=[cc_in[j][256:512, :].opt()],
                            outs=[cc_out3[1][:].opt()])

                # ---------------- phase O for chunk j-1 (collective hidden
                # behind chunk j's projections+attention)
                if j >= 1:
                    phase_o(j - 1)
            phase_o(NCHUNK - 1)

    nc.compile()
    return nc


# ---------------------------------------------------------------- run machinery
class _Runner:
    """Persistent PJRT runner (caches the jitted executable)."""

    def __init__(self, nc, n_cores):
        import jax
        from jax.experimental.shard_map import shard_map
        from jax.sharding import Mesh, PartitionSpec
        from concourse import bass2jax, mybir as mb

        bass2jax.install_neuronx_cc_hook()
        self.jax = jax
        self.n = n_cores
        part_name = (nc.partition_id_tensor.name
                     if nc.partition_id_tensor else None)
        in_names, out_names, out_avals, zero_shapes = [], [], [], []
        for alloc in nc.m.functions[0].allocations:
            if not isinstance(alloc, mb.MemoryLocationSet):
                continue
            name = alloc.memorylocations[0].name
            if alloc.kind == "ExternalInput":
                if name == part_name:
                    continue
                in_names.append(name)
            elif alloc.kind == "ExternalOutput":
                out_names.append(name)
                shape = tuple(alloc.tensor_shape)
                dtype = mb.dt.np(alloc.dtype)
                out_avals.append(jax.core.ShapedArray(shape, dtype))
                zero_shapes.append((shape, dtype))
        self.in_names, self.out_names = in_names, out_names
        self.out_avals, self.zero_shapes = out_avals, zero_shapes
        n_params = len(in_names)
        all_names = tuple(in_names + out_names
                          + ([part_name] if part_name else []))
        donate = tuple(range(n_params, n_params + len(out_names)))

        def _body(*args):
            operands = list(args)
            if part_name is not None:
                operands.append(bass2jax.partition_id_tensor())
            outs = bass2jax._bass_exec_p.bind(
                *operands, out_avals=tuple(out_avals), in_names=all_names,
                out_names=tuple(out_names),
                lowering_input_output_aliases=(),
                sim_require_finite=True, sim_require_nnan=True, nc=nc)
            return tuple(outs)

        devices = jax.devices()[:n_cores]
        self.mesh = Mesh(np.asarray(devices), ("core",))
        in_specs = (PartitionSpec("core"),) * (n_params + len(out_names))
        out_specs = (PartitionSpec("core"),) * len(out_names)
        self.fn = jax.jit(
            shard_map(_body, mesh=self.mesh, in_specs=in_specs,
                      out_specs=out_specs, check_rep=False),
            donate_argnums=donate, keep_unused=True)

    def concat_inputs(self, in_maps):
        return [np.concatenate([np.asarray(m[name]) for m in in_maps], axis=0)
                for name in self.in_names]

    def zeros(self):
        return [np.zeros((self.n * s[0], *s[1:]), d)
                for (s, d) in self.zero_shapes]

    def run(self, in_maps):
        out_arrs = self.fn(*self.concat_inputs(in_maps), *self.zeros())
        return [
            {name: np.asarray(out_arrs[i]).reshape(
                self.n, *self.out_avals[i].shape)[c]
             for i, name in enumerate(self.out_names)}
            for c in range(self.n)
        ]


_STATE = {}


def _get_runner():
    if "runner" not in _STATE:
        nc = build_nc(N_CORES, with_collective=True)
        _STATE["runner"] = _Runner(nc, N_CORES)
    return _STATE["runner"]


def make_in_maps(hidden, wq, wk, wv, wo):
    consts = host_constants()
    b16 = ml_dtypes.bfloat16
    hid2d = np.asarray(hidden, dtype=np.float32).reshape(S, HID)
    # hidH[p, j, t, s] = hidden[512j+s, 128t+p]
    hidH = np.ascontiguousarray(
        hid2d.reshape(NCHUNK, 512, NT, 128).transpose(3, 0, 2, 1)
    ).astype(b16)
    wq = np.asarray(wq, dtype=np.float32)
    wk = np.asarray(wk, dtype=np.float32)
    wv = np.asarray(wv, dtype=np.float32).astype(b16)
    wo = np.asarray(wo, dtype=np.float32).astype(b16)
    # de-interleave head dim of wq/wk (per head) to match the DVE RoPE
    wqp = wq.reshape(HID, NH, HD)[:, :, PERM].reshape(HID, NH * HD).astype(b16)
    wkp = wk.reshape(HID, NKV, HD)[:, :, PERM].reshape(HID, NKV * HD
                                                       ).astype(b16)
    in_maps = []
    for c in range(N_CORES):
        wq_c = wqp[:, DQ * c:DQ * (c + 1)]          # [HID, 512]
        # wqT[p, m, t, d] = wq_c[128t+p, 128m+d]
        wqT = np.ascontiguousarray(
            wq_c.reshape(NT, 128, HPC, 128).transpose(1, 2, 0, 3))
        in_maps.append({
            "hidH": hidH,
            "wqT": wqT,
            "wkT": _tile_w(wkp[:, HD * c:HD * (c + 1)]),
            "wvT": _tile_w(wv[:, HD * c:HD * (c + 1)]),
            "woT": _tile_w(wo[:, DQ * c:DQ * (c + 1)]),
            "cosP": consts["cosP"], "sinS": consts["sinS"],
            "ident": consts["ident"], "ones": consts["ones"],
        })
    return in_maps


def kernel(hidden_states, attention_mask, wq, wk, wv, wo):
    """Full-input entry point: returns [1, S, HID] float32."""
    del attention_mask  # causal mask (-1e9 upper triangle) is hardcoded
    runner = _get_runner()
    in_maps = make_in_maps(hidden_states, wq, wk, wv, wo)
    results = runner.run(in_maps)
    out = np.concatenate([results[c]["out"] for c in range(N_CORES)], axis=1)
    return out.reshape(1, S, HID).astype(np.float32)


# revision 4
# speedup vs baseline: 1.0559x; 1.0559x over previous
"""GQA attention (B=1, S=2048, HID=4096, 32 q-heads / 8 kv-heads, HD=128) on 8
Trainium2 NeuronCores — v8.

v8 changes vs v7 (driven by the v7 trace):
  - o_proj staging loads moved from gpsimd (SWDGE: ~1us issue each plus
    an 8.8us drain right in the tail) to the sync HWDGE queue, which
    after v5 carries only the hidden loads.

v7 changes vs v6 (driven by the v6 trace):
  - phase O split into load (gpsimd DMA issues, emitted right after the
    NEXT chunk's hidden loads so they don't queue behind that chunk's
    attention selects) and matmuls (unchanged position). v6 lost ~30us
    per late chunk waiting for staging loads that were issued only when
    the following attention finished.

v6 changes vs v5 (driven by the v5 trace):
  - All exp-sum accumulation on DVE with a single accumulator (v5 put
    odd tiles on gpsimd, whose slow tensor ops delayed the causal-mask
    selects that gate AV -> recurring 5-7us PE stalls).
  - Broadcast of 1/denom back to bf16 (f32 partition_broadcast was
    1.8us per head on gpsimd).

v5 changes vs v4:
  - DMA queues reassigned so nothing time-critical queues behind a
    stalled write: sync = hidden loads only; scalar = weight loads +
    ao/out writes; gpsimd = o_proj staging loads.
  - Softmax denominator off the PE: exp tiles accumulated into bf16
    running sums, then a 1-row PE matmul per head reduces them (was
    160 full-width PE passes).
  - reciprocal_approx_fast (f32) instead of vector.reciprocal (5x).

v4 changes vs v3 (driven by the v3 trace):
  - All weight/hidden tensors host-pre-tiled to the exact SBUF layout so
    every load DMA is contiguous per partition (v3's strided rearrange
    loads cost 3-10us of descriptor-build EACH on the single sync queue;
    first matmul started at 35us).
  - DMAs spread across the three DGE-capable queues: sync (hidden, ao,
    out), scalar (weight loads, done before exp traffic starts), gpsimd
    (attention-output staging reads for o_proj).
  - Weight loads issued before const loads; first matmul needs only wk +
    first hidden sub-tile.
  - Scores prefetch depth 2 (pS bufs=3) so exp latency never stalls AV.
  - Final chunk's AllGather split into two half-collectives (heads 0-1
    fired early) so the ~30us collective latency hides behind the last
    attention + o_proj of chunk 2.
"""
import math
from contextlib import ExitStack

import numpy as np
import ml_dtypes

import concourse.bass as bass
import concourse.tile as tile
from concourse import bacc, mybir

f32 = mybir.dt.float32
bf16 = mybir.dt.bfloat16

S, HID, NH, NKV, HD = 2048, 4096, 32, 8, 128
N_CORES = 8
HPC = NH // N_CORES           # 4 q heads per core
DQ = HPC * HD                 # 512 q/o columns per core
NCHUNK = S // 512             # 4 sq chunks
NT = HID // 128               # 32 hid tiles
SCALE = 1.0 / math.sqrt(HD)
ROPE_THETA = 10000.0

# even/odd de-interleave permutation of the head dim (applied host-side to
# wq/wk columns and to cos/sin rows); makes rotate_interleaved a 64-partition
# half-swap with sign, which runs on DVE instead of the PE.
PERM = np.concatenate([np.arange(0, HD, 2), np.arange(1, HD, 2)])


# ---------------------------------------------------------------- constants
def host_constants():
    inv = (1.0 / (ROPE_THETA ** (np.arange(0, HD, 2, dtype=np.float32) / HD))
           ).astype(np.float32)
    t = np.arange(S, dtype=np.float32)
    freqs = np.outer(t, inv).astype(np.float32)          # [S, 64]
    emb = np.concatenate([freqs, freqs], axis=1)         # [S, 128]
    cos = np.cos(emb).T                                  # [HD, S]
    sin = np.sin(emb).T
    cosP = np.ascontiguousarray(cos[PERM]).astype(ml_dtypes.bfloat16)
    sinS = sin[PERM].copy()
    sinS[:64] *= -1.0        # top half of roped out = q*cos - q_hi*sin
    sinS = np.ascontiguousarray(sinS).astype(ml_dtypes.bfloat16)
    ident = np.eye(128, dtype=ml_dtypes.bfloat16)
    ones = np.ones((128, 8), dtype=ml_dtypes.bfloat16)
    return {"cosP": cosP, "sinS": sinS, "ident": ident, "ones": ones}


def _tile_w(w):
    """[HID, D] -> [128, NT, D] with w_t[p, t, d] = w[128t+p, d]."""
    D = w.shape[1]
    return np.ascontiguousarray(w.reshape(NT, 128, D).transpose(1, 0, 2))


# ---------------------------------------------------------------- bass build
def build_nc(n_cores=N_CORES, with_collective=True):
    nc = bacc.Bacc("TRN2", target_bir_lowering=False, debug=False,
                   num_devices=n_cores)
    # hidden pre-tiled: hidH[p, j, t, s] = hidden[512j+s, 128t+p]
    hid_d = nc.dram_tensor("hidH", [128, NCHUNK, NT, 512], bf16,
                           kind="ExternalInput").ap()
    # weights pre-tiled: [128, (m,) NT, d]
    wq_d = nc.dram_tensor("wqT", [128, HPC, NT, 128], bf16,
                          kind="ExternalInput").ap()
    wk_d = nc.dram_tensor("wkT", [128, NT, 128], bf16,
                          kind="ExternalInput").ap()
    wv_d = nc.dram_tensor("wvT", [128, NT, 128], bf16,
                          kind="ExternalInput").ap()
    wo_d = nc.dram_tensor("woT", [128, NT, 512], bf16,
                          kind="ExternalInput").ap()
    cos_d = nc.dram_tensor("cosP", [HD, S], bf16, kind="ExternalInput").ap()
    sin_d = nc.dram_tensor("sinS", [HD, S], bf16, kind="ExternalInput").ap()
    ident_d = nc.dram_tensor("ident", [128, 128], bf16,
                             kind="ExternalInput").ap()
    ones_d = nc.dram_tensor("ones", [128, 8], bf16, kind="ExternalInput").ap()
    out_d = nc.dram_tensor("out", [S, DQ], f32, kind="ExternalOutput").ap()

    # per-chunk collective buffers; chunk 3 split into two half-gathers
    cc_in = [nc.dram_tensor(f"cc_in{j}", [DQ, 512], bf16, kind="Internal").ap()
             for j in range(NCHUNK)]
    space = "Shared" if with_collective else None
    kind = "Internal"
    cc_out = [nc.dram_tensor(f"cc_out{j}", [HID, 512], bf16, kind=kind,
                             addr_space=space).ap()
              for j in range(NCHUNK - 1)]
    cc_out3 = [nc.dram_tensor(f"cc_out3{half}", [HID // 2, 512], bf16,
                              kind=kind, addr_space=space).ap()
               for half in ("a", "b")]

    Exp = mybir.ActivationFunctionType.Exp

    with tile.TileContext(nc) as tc, ExitStack() as top:
        constp = top.enter_context(tc.tile_pool(name="const", bufs=1))

        with ExitStack() as ks:
            persist = ks.enter_context(tc.tile_pool(name="persist", bufs=1))
            w_sb = [persist.tile([128, NT, 128], bf16, tag=f"w{m}",
                                 name=f"w{m}") for m in range(HPC + 2)]
            wo_sb = persist.tile([128, NT, 512], bf16, tag="wo")
            kT = persist.tile([128, S], bf16, tag="kT")
            v_sb = persist.tile([128, S], bf16, tag="v_sb")

            # weight loads first (scalar queue): wk, wv needed immediately
            nc.scalar.dma_start(w_sb[0][:], wk_d[:])
            nc.scalar.dma_start(w_sb[1][:], wv_d[:])
            for m in range(HPC):
                nc.scalar.dma_start(w_sb[2 + m][:], wq_d[:, m])

            ident = constp.tile([128, 128], bf16, tag="ident")
            nc.scalar.dma_start(ident[:], ident_d[:])
            ones = constp.tile([128, 8], bf16, tag="ones")
            nc.scalar.dma_start(ones[:], ones_d[:])
            cos_sb = constp.tile([128, S], bf16, tag="cos")
            nc.scalar.dma_start(cos_sb[:], cos_d[:])
            sin_sb = constp.tile([128, S], bf16, tag="sin")
            nc.scalar.dma_start(sin_sb[:], sin_d[:])

            hTap = ks.enter_context(tc.tile_pool(name="hTa", bufs=2))
            hTbp = ks.enter_context(tc.tile_pool(name="hTb", bufs=1))
            qTp = ks.enter_context(tc.tile_pool(name="qT", bufs=2))
            tmpp = ks.enter_context(tc.tile_pool(name="tmp", bufs=2))
            rawp = ks.enter_context(tc.tile_pool(name="raw", bufs=1))
            Ep = ks.enter_context(tc.tile_pool(name="E", bufs=4))
            smp = ks.enter_context(tc.tile_pool(name="sm", bufs=2))
            rcp = ks.enter_context(tc.tile_pool(name="rc", bufs=1))
            aop = ks.enter_context(tc.tile_pool(name="ao", bufs=2))
            aTp = ks.enter_context(tc.tile_pool(name="aT", bufs=1))
            osbp = ks.enter_context(tc.tile_pool(name="osb", bufs=1))
            # PSUM: acc(2) + vtr(1) + pS(3) + pO(1) + pD(1) = 8 banks
            paccp = ks.enter_context(
                tc.tile_pool(name="pacc", bufs=2, space="PSUM"))
            pvtrp = ks.enter_context(
                tc.tile_pool(name="pvtr", bufs=1, space="PSUM"))
            pSp = ks.enter_context(
                tc.tile_pool(name="pS", bufs=3, space="PSUM"))
            pOp = ks.enter_context(
                tc.tile_pool(name="pO", bufs=1, space="PSUM"))
            pDp = ks.enter_context(
                tc.tile_pool(name="pD", bufs=1, space="PSUM"))

            def rope_dve(acc, cos_c, sinS_c, dest):
                """dest = acc*cos + halfswap(acc)*sin (all [128, 512])."""
                t1 = tmpp.tile([128, 512], bf16, tag="t1")
                nc.vector.tensor_mul(t1[:], acc[:], cos_c)
                t2 = tmpp.tile([128, 512], bf16, tag="t2")
                nc.vector.tensor_mul(t2[0:64, :], acc[64:128, :], sinS_c[0:64])
                nc.vector.tensor_mul(t2[64:128, :], acc[0:64, :],
                                     sinS_c[64:128])
                nc.vector.tensor_add(dest, t1[:], t2[:])

            def phase_o_load_a(jo):
                """issue the o_proj staging DMAs for chunk jo (first half)."""
                aTa = aTp.tile([128, NT // 2, 512], bf16, tag="aTa")
                src_t = (cc_out[jo] if jo < NCHUNK - 1 else cc_out3[0])
                a3 = src_t.rearrange("(t p) s -> p t s", p=128)
                for g in range(2):
                    nc.sync.dma_start(aTa[:, 8 * g:8 * (g + 1), :],
                                      a3[:, 8 * g:8 * (g + 1), :])
                return aTa

            def phase_o_load_b(jo):
                """issue the o_proj staging DMAs for chunk jo (second half)."""
                aTb = aTp.tile([128, NT // 2, 512], bf16, tag="aTb")
                if jo < NCHUNK - 1:
                    a3 = cc_out[jo].rearrange("(t p) s -> p t s", p=128)
                    for g in range(2):
                        nc.sync.dma_start(
                            aTb[:, 8 * g:8 * (g + 1), :],
                            a3[:, 16 + 8 * g:16 + 8 * (g + 1), :])
                else:
                    a3b = cc_out3[1].rearrange("(t p) s -> p t s", p=128)
                    for g in range(2):
                        nc.sync.dma_start(aTb[:, 8 * g:8 * (g + 1), :],
                                          a3b[:, 8 * g:8 * (g + 1), :])
                return aTb

            def phase_o_mm(jo, aTa, aTb):
                """o_proj matmuls for sq chunk jo."""
                if jo < NCHUNK - 1:
                    # accumulation order: t = 0..31; tile t lives in
                    # aTa[t] for t<16 else aTb[t-16]
                    order = [(t, (aTa, t) if t < 16 else (aTb, t - 16))
                             for t in range(NT)]
                else:
                    # half a holds heads 0-1 of each core: global hid tile
                    # t = 4c+h -> (aTa, 2c+h) for h<2, (aTb, 2c+h-2) else
                    order = []
                    for c in range(8):
                        for h in range(2):
                            order.append((4 * c + h, (aTa, 2 * c + h)))
                    for c in range(8):
                        for h in range(2, 4):
                            order.append((4 * c + h, (aTb, 2 * c + h - 2)))
                for b in range(4):
                    acc = paccp.tile([128, 512], f32, tag="acc")
                    for n, (t, (src, idx)) in enumerate(order):
                        nc.tensor.matmul(
                            acc[:], src[:, idx, 128 * b:128 * (b + 1)],
                            wo_sb[:, t, :],
                            start=(n == 0), stop=(n == NT - 1))
                    o_sb = osbp.tile([128, 512], f32, tag="osb")
                    nc.scalar.copy(o_sb[:], acc[:])
                    sqt = 4 * jo + b
                    nc.scalar.dma_start(out_d[128 * sqt:128 * (sqt + 1), :],
                                        o_sb[:])

            def load_hT(j):
                hTa = hTap.tile([128, NT // 2, 512], bf16, tag="hTa")
                hTb = hTbp.tile([128, NT // 2, 512], bf16, tag="hTb")
                for g in range(2):
                    nc.sync.dma_start(hTa[:, 8 * g:8 * (g + 1), :],
                                      hid_d[:, j, 8 * g:8 * (g + 1), :])
                for g in range(2):
                    nc.sync.dma_start(hTb[:, 8 * g:8 * (g + 1), :],
                                      hid_d[:, j, 16 + 8 * g:24 + 8 * g, :])
                return hTa, hTb

            o_tiles = {}
            hT_next = load_hT(0)
            for j in range(NCHUNK):
                # ---------------- phase P for chunk j
                hTa, hTb = hT_next
                if j >= 1:
                    # stage chunk j-1's gathered outputs: issued now (sync
                    # queue, nothing time-critical behind), data dep
                    # (gather j-1) resolves during this chunk
                    o_tiles[j - 1] = (phase_o_load_a(j - 1),
                                      phase_o_load_b(j - 1))

                def hT(t):
                    return hTa[:, t, :] if t < 16 else hTb[:, t - 16, :]

                cos_c = cos_sb[:, 512 * j:512 * (j + 1)]
                sin_c = sin_sb[:, 512 * j:512 * (j + 1)]
                if j == 1:   # prefetch wo for phase O during chunk 1
                    for g in range(4):
                        nc.scalar.dma_start(wo_sb[:, 8 * g:8 * (g + 1), :],
                                            wo_d[:, 8 * g:8 * (g + 1), :])

                qT = qTp.tile([128, HPC, 512], bf16, tag="qT")
                # m order: k, v, q0..q3 so attention can start earliest
                accs = {}
                for m in range(HPC + 2):
                    acc = paccp.tile([128, 512], f32, tag="acc")
                    accs[m] = acc
                    for t in range(NT):
                        nc.tensor.matmul(
                            acc[:], w_sb[m][:, t, :], hT(t),
                            start=(t == 0), stop=(t == NT - 1))
                    if m == 0:
                        rope_dve(acc, cos_c, sin_c,
                                 kT[:, 512 * j:512 * (j + 1)])
                    elif m == 1:
                        # v: drain now (ACT), transpose staggered after q0
                        raw = rawp.tile([128, 512], bf16, tag="raw")
                        nc.scalar.copy(raw[:], acc[:])
                        accs["raw_v"] = raw
                    else:
                        rope_dve(acc, cos_c, sin_c, qT[:, m - 2, :])
                        if m == 2:
                            # v transpose (PE) staggered behind q0's matmuls
                            raw = accs["raw_v"]
                            ps = pvtrp.tile([128, 512], bf16, tag="vtr")
                            for tt in range(4):
                                nc.tensor.matmul(
                                    ps[:, 128 * tt:128 * (tt + 1)],
                                    raw[:, 128 * tt:128 * (tt + 1)],
                                    ident[:], is_transpose=True,
                                    start=(tt == 0), stop=(tt == 3))
                            nc.vector.tensor_copy(
                                v_sb[:, 512 * j:512 * (j + 1)], ps[:])

                # ---------------- phase A for chunk j (all heads)
                for h in range(HPC):
                    ni = 4 * j + 4
                    acc_o = pOp.tile([128, 512], f32, tag="pO")
                    acc_d = pDp.tile([1, 512], f32, tag="pD")

                    def _delta(i):
                        return max(0, 128 * i - 512 * j)

                    def scores_mm(i):
                        d = _delta(i)
                        ps = pSp.tile([128, 512], f32, tag="pS")
                        nc.tensor.matmul(ps[:, d:],
                                         kT[:, 128 * i:128 * (i + 1)],
                                         qT[:, h, d:], start=True, stop=True)
                        return ps

                    pss = [scores_mm(0)]
                    if ni > 1:
                        pss.append(scores_mm(1))
                    # exp-sum accumulator on DVE (replaces a full-width
                    # PE pass per sk-tile); tile 0 is always full-width
                    dsA = smp.tile([128, 512], bf16, tag="dsA")
                    for i in range(ni):
                        d0 = _delta(i)
                        w = 512 - d0
                        E = Ep.tile([128, 512], bf16, tag="E")
                        nc.scalar.activation(E[:, d0:], pss[i][:, d0:], Exp,
                                             scale=SCALE)
                        if i >= 4 * j:   # diagonal-crossing tile: mask
                            nc.gpsimd.affine_select(
                                E[:, d0:], E[:, d0:], pattern=[[1, w]],
                                compare_op=mybir.AluOpType.is_ge,
                                fill=0.0, base=0,
                                channel_multiplier=-1)
                        if i + 2 < ni:
                            pss.append(scores_mm(i + 2))
                        nc.tensor.matmul(acc_o[:, d0:],
                                         v_sb[:, 128 * i:128 * (i + 1)],
                                         E[:, d0:], start=(i == 0),
                                         stop=(i == ni - 1))
                        with nc.allow_low_precision(reason="softmax denom"):
                            if i == 0:
                                nc.vector.tensor_copy(dsA[:], E[:])
                            else:
                                nc.vector.tensor_add(dsA[:, d0:], dsA[:, d0:],
                                                     E[:, d0:])
                    nc.tensor.matmul(acc_d[:], ones[:, 0:1], dsA[:],
                                     start=True, stop=True)
                    recip = rcp.tile([1, 512], f32, tag="recip")
                    nc.vector.reciprocal_approx_fast(recip[:], acc_d[:])
                    recip_bf = rcp.tile([1, 512], bf16, tag="recipb")
                    with nc.allow_low_precision(reason="softmax denom"):
                        nc.vector.tensor_copy(recip_bf[:], recip[:])
                    bc = smp.tile([128, 512], bf16, tag="bc")
                    nc.gpsimd.partition_broadcast(bc[:], recip_bf[:])
                    ao = aop.tile([128, 512], bf16, tag="ao")
                    nc.vector.tensor_mul(ao[:], acc_o[:], bc[:])
                    nc.scalar.dma_start(
                        cc_in[j][128 * h:128 * (h + 1), :], ao[:])

                    # last chunk: fire the first half-gather once heads 0-1
                    # are out, so its latency hides behind heads 2-3
                    if with_collective and j == NCHUNK - 1 and h == 1:
                        nc.gpsimd.collective_compute(
                            "AllGather", mybir.AluOpType.bypass,
                            replica_groups=[list(range(n_cores))],
                            ins=[cc_in[j][0:256, :].opt()],
                            outs=[cc_out3[0][:].opt()])

                # ---------------- AllGather for chunk j
                if with_collective:
                    if j < NCHUNK - 1:
                        nc.gpsimd.collective_compute(
                            "AllGather", mybir.AluOpType.bypass,
                            replica_groups=[list(range(n_cores))],
                            ins=[cc_in[j][:].opt()], outs=[cc_out[j][:].opt()])
                    else:
                        nc.gpsimd.collective_compute(
                            "AllGather", mybir.AluOpType.bypass,
                            replica_groups=[list(range(n_cores))],
                            ins=[cc_in[j][256:512, :].opt()],
                            outs=[cc_out3[1][:].opt()])

                if j < NCHUNK - 1:
                    # next chunk's hidden loads BEFORE the aT issues that
                    # wait on collectives (sync queue is in-order)
                    hT_next = load_hT(j + 1)
                else:
                    # both staging loads after the last trigger: with bufs=1
                    # tiles they carry WAR deps on O2's reads, so anything
                    # emitted behind them would deadlock A3
                    o_tiles["3a"] = phase_o_load_a(NCHUNK - 1)
                    o_tiles["3b"] = phase_o_load_b(NCHUNK - 1)

                # ---------------- phase O for chunk j-1 (collective hidden
                # behind chunk j's projections+attention)
                if j >= 1:
                    phase_o_mm(j - 1, *o_tiles.pop(j - 1))
            phase_o_mm(NCHUNK - 1, o_tiles.pop("3a"), o_tiles.pop("3b"))

    nc.compile()
    return nc


# ---------------------------------------------------------------- run machinery
class _Runner:
    """Persistent PJRT runner (caches the jitted executable)."""

    def __init__(self, nc, n_cores):
        import jax
        from jax.experimental.shard_map import shard_map
        from jax.sharding import Mesh, PartitionSpec
        from concourse import bass2jax, mybir as mb

        bass2jax.install_neuronx_cc_hook()
        self.jax = jax
        self.n = n_cores
        part_name = (nc.partition_id_tensor.name
                     if nc.partition_id_tensor else None)
        in_names, out_names, out_avals, zero_shapes = [], [], [], []
        for alloc in nc.m.functions[0].allocations:
            if not isinstance(alloc, mb.MemoryLocationSet):
                continue
            name = alloc.memorylocations[0].name
            if alloc.kind == "ExternalInput":
                if name == part_name:
                    continue
                in_names.append(name)
            elif alloc.kind == "ExternalOutput":
                out_names.append(name)
                shape = tuple(alloc.tensor_shape)
                dtype = mb.dt.np(alloc.dtype)
                out_avals.append(jax.core.ShapedArray(shape, dtype))
                zero_shapes.append((shape, dtype))
        self.in_names, self.out_names = in_names, out_names
        self.out_avals, self.zero_shapes = out_avals, zero_shapes
        n_params = len(in_names)
        all_names = tuple(in_names + out_names
                          + ([part_name] if part_name else []))
        donate = tuple(range(n_params, n_params + len(out_names)))

        def _body(*args):
            operands = list(args)
            if part_name is not None:
                operands.append(bass2jax.partition_id_tensor())
            outs = bass2jax._bass_exec_p.bind(
                *operands, out_avals=tuple(out_avals), in_names=all_names,
                out_names=tuple(out_names),
                lowering_input_output_aliases=(),
                sim_require_finite=True, sim_require_nnan=True, nc=nc)
            return tuple(outs)

        devices = jax.devices()[:n_cores]
        self.mesh = Mesh(np.asarray(devices), ("core",))
        in_specs = (PartitionSpec("core"),) * (n_params + len(out_names))
        out_specs = (PartitionSpec("core"),) * len(out_names)
        self.fn = jax.jit(
            shard_map(_body, mesh=self.mesh, in_specs=in_specs,
                      out_specs=out_specs, check_rep=False),
            donate_argnums=donate, keep_unused=True)

    def concat_inputs(self, in_maps):
        return [np.concatenate([np.asarray(m[name]) for m in in_maps], axis=0)
                for name in self.in_names]

    def zeros(self):
        return [np.zeros((self.n * s[0], *s[1:]), d)
                for (s, d) in self.zero_shapes]

    def run(self, in_maps):
        out_arrs = self.fn(*self.concat_inputs(in_maps), *self.zeros())
        return [
            {name: np.asarray(out_arrs[i]).reshape(
                self.n, *self.out_avals[i].shape)[c]
             for i, name in enumerate(self.out_names)}
            for c in range(self.n)
        ]


_STATE = {}


def _get_runner():
    if "runner" not in _STATE:
        nc = build_nc(N_CORES, with_collective=True)
        _STATE["runner"] = _Runner(nc, N_CORES)
    return _STATE["runner"]


def make_in_maps(hidden, wq, wk, wv, wo):
    consts = host_constants()
    b16 = ml_dtypes.bfloat16
    hid2d = np.asarray(hidden, dtype=np.float32).reshape(S, HID)
    # hidH[p, j, t, s] = hidden[512j+s, 128t+p]
    hidH = np.ascontiguousarray(
        hid2d.reshape(NCHUNK, 512, NT, 128).transpose(3, 0, 2, 1)
    ).astype(b16)
    wq = np.asarray(wq, dtype=np.float32)
    wk = np.asarray(wk, dtype=np.float32)
    wv = np.asarray(wv, dtype=np.float32).astype(b16)
    wo = np.asarray(wo, dtype=np.float32).astype(b16)
    # de-interleave head dim of wq/wk (per head) to match the DVE RoPE
    wqp = wq.reshape(HID, NH, HD)[:, :, PERM].reshape(HID, NH * HD).astype(b16)
    wkp = wk.reshape(HID, NKV, HD)[:, :, PERM].reshape(HID, NKV * HD
                                                       ).astype(b16)
    in_maps = []
    for c in range(N_CORES):
        wq_c = wqp[:, DQ * c:DQ * (c + 1)]          # [HID, 512]
        # wqT[p, m, t, d] = wq_c[128t+p, 128m+d]
        wqT = np.ascontiguousarray(
            wq_c.reshape(NT, 128, HPC, 128).transpose(1, 2, 0, 3))
        in_maps.append({
            "hidH": hidH,
            "wqT": wqT,
            "wkT": _tile_w(wkp[:, HD * c:HD * (c + 1)]),
            "wvT": _tile_w(wv[:, HD * c:HD * (c + 1)]),
            "woT": _tile_w(wo[:, DQ * c:DQ * (c + 1)]),
            "cosP": consts["cosP"], "sinS": consts["sinS"],
            "ident": consts["ident"], "ones": consts["ones"],
        })
    return in_maps


def kernel(hidden_states, attention_mask, wq, wk, wv, wo):
    """Full-input entry point: returns [1, S, HID] float32."""
    del attention_mask  # causal mask (-1e9 upper triangle) is hardcoded
    runner = _get_runner()
    in_maps = make_in_maps(hidden_states, wq, wk, wv, wo)
    results = runner.run(in_maps)
    out = np.concatenate([results[c]["out"] for c in range(N_CORES)], axis=1)
    return out.reshape(1, S, HID).astype(np.float32)


# revision 5
# speedup vs baseline: 1.0664x; 1.0100x over previous
"""GQA attention (B=1, S=2048, HID=4096, 32 q-heads / 8 kv-heads, HD=128) on 8
Trainium2 NeuronCores — v10.

v10 changes vs v8 (driven by the v8 trace):
  - wk (the first-needed weight) loaded in four 8-tile sub-DMAs: the
    first matmul was gated by wk's full 1MB transfer through startup
    HBM contention (first matmul at 24.4us; only ~256KB is needed).

v8 changes vs v7 (driven by the v7 trace):
  - o_proj staging loads moved from gpsimd (SWDGE: ~1us issue each plus
    an 8.8us drain right in the tail) to the sync HWDGE queue, which
    after v5 carries only the hidden loads.

v7 changes vs v6 (driven by the v6 trace):
  - phase O split into load (gpsimd DMA issues, emitted right after the
    NEXT chunk's hidden loads so they don't queue behind that chunk's
    attention selects) and matmuls (unchanged position). v6 lost ~30us
    per late chunk waiting for staging loads that were issued only when
    the following attention finished.

v6 changes vs v5 (driven by the v5 trace):
  - All exp-sum accumulation on DVE with a single accumulator (v5 put
    odd tiles on gpsimd, whose slow tensor ops delayed the causal-mask
    selects that gate AV -> recurring 5-7us PE stalls).
  - Broadcast of 1/denom back to bf16 (f32 partition_broadcast was
    1.8us per head on gpsimd).

v5 changes vs v4:
  - DMA queues reassigned so nothing time-critical queues behind a
    stalled write: sync = hidden loads only; scalar = weight loads +
    ao/out writes; gpsimd = o_proj staging loads.
  - Softmax denominator off the PE: exp tiles accumulated into bf16
    running sums, then a 1-row PE matmul per head reduces them (was
    160 full-width PE passes).
  - reciprocal_approx_fast (f32) instead of vector.reciprocal (5x).

v4 changes vs v3 (driven by the v3 trace):
  - All weight/hidden tensors host-pre-tiled to the exact SBUF layout so
    every load DMA is contiguous per partition (v3's strided rearrange
    loads cost 3-10us of descriptor-build EACH on the single sync queue;
    first matmul started at 35us).
  - DMAs spread across the three DGE-capable queues: sync (hidden, ao,
    out), scalar (weight loads, done before exp traffic starts), gpsimd
    (attention-output staging reads for o_proj).
  - Weight loads issued before const loads; first matmul needs only wk +
    first hidden sub-tile.
  - Scores prefetch depth 2 (pS bufs=3) so exp latency never stalls AV.
  - Final chunk's AllGather split into two half-collectives (heads 0-1
    fired early) so the ~30us collective latency hides behind the last
    attention + o_proj of chunk 2.
"""
import math
from contextlib import ExitStack

import numpy as np
import ml_dtypes

import concourse.bass as bass
import concourse.tile as tile
from concourse import bacc, mybir

f32 = mybir.dt.float32
bf16 = mybir.dt.bfloat16

S, HID, NH, NKV, HD = 2048, 4096, 32, 8, 128
N_CORES = 8
HPC = NH // N_CORES           # 4 q heads per core
DQ = HPC * HD                 # 512 q/o columns per core
NCHUNK = S // 512             # 4 sq chunks
NT = HID // 128               # 32 hid tiles
SCALE = 1.0 / math.sqrt(HD)
ROPE_THETA = 10000.0

# even/odd de-interleave permutation of the head dim (applied host-side to
# wq/wk columns and to cos/sin rows); makes rotate_interleaved a 64-partition
# half-swap with sign, which runs on DVE instead of the PE.
PERM = np.concatenate([np.arange(0, HD, 2), np.arange(1, HD, 2)])


# ---------------------------------------------------------------- constants
def host_constants():
    inv = (1.0 / (ROPE_THETA ** (np.arange(0, HD, 2, dtype=np.float32) / HD))
           ).astype(np.float32)
    t = np.arange(S, dtype=np.float32)
    freqs = np.outer(t, inv).astype(np.float32)          # [S, 64]
    emb = np.concatenate([freqs, freqs], axis=1)         # [S, 128]
    cos = np.cos(emb).T                                  # [HD, S]
    sin = np.sin(emb).T
    cosP = np.ascontiguousarray(cos[PERM]).astype(ml_dtypes.bfloat16)
    sinS = sin[PERM].copy()
    sinS[:64] *= -1.0        # top half of roped out = q*cos - q_hi*sin
    sinS = np.ascontiguousarray(sinS).astype(ml_dtypes.bfloat16)
    ident = np.eye(128, dtype=ml_dtypes.bfloat16)
    ones = np.ones((128, 8), dtype=ml_dtypes.bfloat16)
    return {"cosP": cosP, "sinS": sinS, "ident": ident, "ones": ones}


def _tile_w(w):
    """[HID, D] -> [128, NT, D] with w_t[p, t, d] = w[128t+p, d]."""
    D = w.shape[1]
    return np.ascontiguousarray(w.reshape(NT, 128, D).transpose(1, 0, 2))


# ---------------------------------------------------------------- bass build
def build_nc(n_cores=N_CORES, with_collective=True):
    nc = bacc.Bacc("TRN2", target_bir_lowering=False, debug=False,
                   num_devices=n_cores)
    # hidden pre-tiled: hidH[p, j, t, s] = hidden[512j+s, 128t+p]
    hid_d = nc.dram_tensor("hidH", [128, NCHUNK, NT, 512], bf16,
                           kind="ExternalInput").ap()
    # weights pre-tiled: [128, (m,) NT, d]
    wq_d = nc.dram_tensor("wqT", [128, HPC, NT, 128], bf16,
                          kind="ExternalInput").ap()
    wk_d = nc.dram_tensor("wkT", [128, NT, 128], bf16,
                          kind="ExternalInput").ap()
    wv_d = nc.dram_tensor("wvT", [128, NT, 128], bf16,
                          kind="ExternalInput").ap()
    wo_d = nc.dram_tensor("woT", [128, NT, 512], bf16,
                          kind="ExternalInput").ap()
    cos_d = nc.dram_tensor("cosP", [HD, S], bf16, kind="ExternalInput").ap()
    sin_d = nc.dram_tensor("sinS", [HD, S], bf16, kind="ExternalInput").ap()
    ident_d = nc.dram_tensor("ident", [128, 128], bf16,
                             kind="ExternalInput").ap()
    ones_d = nc.dram_tensor("ones", [128, 8], bf16, kind="ExternalInput").ap()
    out_d = nc.dram_tensor("out", [S, DQ], f32, kind="ExternalOutput").ap()

    # per-chunk collective buffers; chunk 3 split into two half-gathers
    cc_in = [nc.dram_tensor(f"cc_in{j}", [DQ, 512], bf16, kind="Internal").ap()
             for j in range(NCHUNK)]
    space = "Shared" if with_collective else None
    kind = "Internal"
    cc_out = [nc.dram_tensor(f"cc_out{j}", [HID, 512], bf16, kind=kind,
                             addr_space=space).ap()
              for j in range(NCHUNK - 1)]
    cc_out3 = [nc.dram_tensor(f"cc_out3{half}", [HID // 2, 512], bf16,
                              kind=kind, addr_space=space).ap()
               for half in ("a", "b")]

    Exp = mybir.ActivationFunctionType.Exp

    with tile.TileContext(nc) as tc, ExitStack() as top:
        constp = top.enter_context(tc.tile_pool(name="const", bufs=1))

        with ExitStack() as ks:
            persist = ks.enter_context(tc.tile_pool(name="persist", bufs=1))
            w_sb = [persist.tile([128, NT, 128], bf16, tag=f"w{m}",
                                 name=f"w{m}") for m in range(HPC + 2)]
            wo_sb = persist.tile([128, NT, 512], bf16, tag="wo")
            kT = persist.tile([128, S], bf16, tag="kT")
            v_sb = persist.tile([128, S], bf16, tag="v_sb")

            # weight loads first (scalar queue): wk, wv needed immediately;
            # wk in sub-DMAs so the first projection starts on tile 0
            for g in range(4):
                nc.scalar.dma_start(w_sb[0][:, 8 * g:8 * (g + 1), :],
                                    wk_d[:, 8 * g:8 * (g + 1), :])
            nc.scalar.dma_start(w_sb[1][:], wv_d[:])
            for m in range(HPC):
                nc.scalar.dma_start(w_sb[2 + m][:], wq_d[:, m])

            ident = constp.tile([128, 128], bf16, tag="ident")
            nc.scalar.dma_start(ident[:], ident_d[:])
            ones = constp.tile([128, 8], bf16, tag="ones")
            nc.scalar.dma_start(ones[:], ones_d[:])
            cos_sb = constp.tile([128, S], bf16, tag="cos")
            nc.scalar.dma_start(cos_sb[:], cos_d[:])
            sin_sb = constp.tile([128, S], bf16, tag="sin")
            nc.scalar.dma_start(sin_sb[:], sin_d[:])

            hTap = ks.enter_context(tc.tile_pool(name="hTa", bufs=2))
            hTbp = ks.enter_context(tc.tile_pool(name="hTb", bufs=1))
            qTp = ks.enter_context(tc.tile_pool(name="qT", bufs=2))
            tmpp = ks.enter_context(tc.tile_pool(name="tmp", bufs=2))
            rawp = ks.enter_context(tc.tile_pool(name="raw", bufs=1))
            Ep = ks.enter_context(tc.tile_pool(name="E", bufs=4))
            smp = ks.enter_context(tc.tile_pool(name="sm", bufs=2))
            rcp = ks.enter_context(tc.tile_pool(name="rc", bufs=1))
            aop = ks.enter_context(tc.tile_pool(name="ao", bufs=2))
            aTp = ks.enter_context(tc.tile_pool(name="aT", bufs=1))
            osbp = ks.enter_context(tc.tile_pool(name="osb", bufs=1))
            # PSUM: acc(2) + vtr(1) + pS(3) + pO(1) + pD(1) = 8 banks
            paccp = ks.enter_context(
                tc.tile_pool(name="pacc", bufs=2, space="PSUM"))
            pvtrp = ks.enter_context(
                tc.tile_pool(name="pvtr", bufs=1, space="PSUM"))
            pSp = ks.enter_context(
                tc.tile_pool(name="pS", bufs=3, space="PSUM"))
            pOp = ks.enter_context(
                tc.tile_pool(name="pO", bufs=1, space="PSUM"))
            pDp = ks.enter_context(
                tc.tile_pool(name="pD", bufs=1, space="PSUM"))

            def rope_dve(acc, cos_c, sinS_c, dest):
                """dest = acc*cos + halfswap(acc)*sin (all [128, 512])."""
                t1 = tmpp.tile([128, 512], bf16, tag="t1")
                nc.vector.tensor_mul(t1[:], acc[:], cos_c)
                t2 = tmpp.tile([128, 512], bf16, tag="t2")
                nc.vector.tensor_mul(t2[0:64, :], acc[64:128, :], sinS_c[0:64])
                nc.vector.tensor_mul(t2[64:128, :], acc[0:64, :],
                                     sinS_c[64:128])
                nc.vector.tensor_add(dest, t1[:], t2[:])

            def phase_o_load_a(jo):
                """issue the o_proj staging DMAs for chunk jo (first half)."""
                aTa = aTp.tile([128, NT // 2, 512], bf16, tag="aTa")
                src_t = (cc_out[jo] if jo < NCHUNK - 1 else cc_out3[0])
                a3 = src_t.rearrange("(t p) s -> p t s", p=128)
                for g in range(2):
                    nc.sync.dma_start(aTa[:, 8 * g:8 * (g + 1), :],
                                      a3[:, 8 * g:8 * (g + 1), :])
                return aTa

            def phase_o_load_b(jo):
                """issue the o_proj staging DMAs for chunk jo (second half)."""
                aTb = aTp.tile([128, NT // 2, 512], bf16, tag="aTb")
                if jo < NCHUNK - 1:
                    a3 = cc_out[jo].rearrange("(t p) s -> p t s", p=128)
                    for g in range(2):
                        nc.sync.dma_start(
                            aTb[:, 8 * g:8 * (g + 1), :],
                            a3[:, 16 + 8 * g:16 + 8 * (g + 1), :])
                else:
                    a3b = cc_out3[1].rearrange("(t p) s -> p t s", p=128)
                    for g in range(2):
                        nc.sync.dma_start(aTb[:, 8 * g:8 * (g + 1), :],
                                          a3b[:, 8 * g:8 * (g + 1), :])
                return aTb

            def phase_o_mm(jo, aTa, aTb):
                """o_proj matmuls for sq chunk jo."""
                if jo < NCHUNK - 1:
                    # accumulation order: t = 0..31; tile t lives in
                    # aTa[t] for t<16 else aTb[t-16]
                    order = [(t, (aTa, t) if t < 16 else (aTb, t - 16))
                             for t in range(NT)]
                else:
                    # half a holds heads 0-1 of each core: global hid tile
                    # t = 4c+h -> (aTa, 2c+h) for h<2, (aTb, 2c+h-2) else
                    order = []
                    for c in range(8):
                        for h in range(2):
                            order.append((4 * c + h, (aTa, 2 * c + h)))
                    for c in range(8):
                        for h in range(2, 4):
                            order.append((4 * c + h, (aTb, 2 * c + h - 2)))
                for b in range(4):
                    acc = paccp.tile([128, 512], f32, tag="acc")
                    for n, (t, (src, idx)) in enumerate(order):
                        nc.tensor.matmul(
                            acc[:], src[:, idx, 128 * b:128 * (b + 1)],
                            wo_sb[:, t, :],
                            start=(n == 0), stop=(n == NT - 1))
                    o_sb = osbp.tile([128, 512], f32, tag="osb")
                    nc.scalar.copy(o_sb[:], acc[:])
                    sqt = 4 * jo + b
                    nc.scalar.dma_start(out_d[128 * sqt:128 * (sqt + 1), :],
                                        o_sb[:])

            def load_hT(j):
                hTa = hTap.tile([128, NT // 2, 512], bf16, tag="hTa")
                hTb = hTbp.tile([128, NT // 2, 512], bf16, tag="hTb")
                for g in range(2):
                    nc.sync.dma_start(hTa[:, 8 * g:8 * (g + 1), :],
                                      hid_d[:, j, 8 * g:8 * (g + 1), :])
                for g in range(2):
                    nc.sync.dma_start(hTb[:, 8 * g:8 * (g + 1), :],
                                      hid_d[:, j, 16 + 8 * g:24 + 8 * g, :])
                return hTa, hTb

            o_tiles = {}
            hT_next = load_hT(0)
            for j in range(NCHUNK):
                # ---------------- phase P for chunk j
                hTa, hTb = hT_next
                if j >= 1:
                    # stage chunk j-1's gathered outputs: issued now (sync
                    # queue, nothing time-critical behind), data dep
                    # (gather j-1) resolves during this chunk
                    o_tiles[j - 1] = (phase_o_load_a(j - 1),
                                      phase_o_load_b(j - 1))

                def hT(t):
                    return hTa[:, t, :] if t < 16 else hTb[:, t - 16, :]

                cos_c = cos_sb[:, 512 * j:512 * (j + 1)]
                sin_c = sin_sb[:, 512 * j:512 * (j + 1)]
                if j == 1:   # prefetch wo for phase O during chunk 1
                    for g in range(4):
                        nc.scalar.dma_start(wo_sb[:, 8 * g:8 * (g + 1), :],
                                            wo_d[:, 8 * g:8 * (g + 1), :])

                qT = qTp.tile([128, HPC, 512], bf16, tag="qT")
                # m order: k, v, q0..q3 so attention can start earliest
                accs = {}
                for m in range(HPC + 2):
                    acc = paccp.tile([128, 512], f32, tag="acc")
                    accs[m] = acc
                    for t in range(NT):
                        nc.tensor.matmul(
                            acc[:], w_sb[m][:, t, :], hT(t),
                            start=(t == 0), stop=(t == NT - 1))
                    if m == 0:
                        rope_dve(acc, cos_c, sin_c,
                                 kT[:, 512 * j:512 * (j + 1)])
                    elif m == 1:
                        # v: drain now (ACT), transpose staggered after q0
                        raw = rawp.tile([128, 512], bf16, tag="raw")
                        nc.scalar.copy(raw[:], acc[:])
                        accs["raw_v"] = raw
                    else:
                        rope_dve(acc, cos_c, sin_c, qT[:, m - 2, :])
                        if m == 2:
                            # v transpose (PE) staggered behind q0's matmuls
                            raw = accs["raw_v"]
                            ps = pvtrp.tile([128, 512], bf16, tag="vtr")
                            for tt in range(4):
                                nc.tensor.matmul(
                                    ps[:, 128 * tt:128 * (tt + 1)],
                                    raw[:, 128 * tt:128 * (tt + 1)],
                                    ident[:], is_transpose=True,
                                    start=(tt == 0), stop=(tt == 3))
                            nc.vector.tensor_copy(
                                v_sb[:, 512 * j:512 * (j + 1)], ps[:])

                # ---------------- phase A for chunk j (all heads)
                for h in range(HPC):
                    ni = 4 * j + 4
                    acc_o = pOp.tile([128, 512], f32, tag="pO")
                    acc_d = pDp.tile([1, 512], f32, tag="pD")

                    def _delta(i):
                        return max(0, 128 * i - 512 * j)

                    def scores_mm(i):
                        d = _delta(i)
                        ps = pSp.tile([128, 512], f32, tag="pS")
                        nc.tensor.matmul(ps[:, d:],
                                         kT[:, 128 * i:128 * (i + 1)],
                                         qT[:, h, d:], start=True, stop=True)
                        return ps

                    pss = [scores_mm(0)]
                    if ni > 1:
                        pss.append(scores_mm(1))
                    # exp-sum accumulator on DVE (replaces a full-width
                    # PE pass per sk-tile); tile 0 is always full-width
                    dsA = smp.tile([128, 512], bf16, tag="dsA")
                    for i in range(ni):
                        d0 = _delta(i)
                        w = 512 - d0
                        E = Ep.tile([128, 512], bf16, tag="E")
                        nc.scalar.activation(E[:, d0:], pss[i][:, d0:], Exp,
                                             scale=SCALE)
                        if i >= 4 * j:   # diagonal-crossing tile: mask
                            nc.gpsimd.affine_select(
                                E[:, d0:], E[:, d0:], pattern=[[1, w]],
                                compare_op=mybir.AluOpType.is_ge,
                                fill=0.0, base=0,
                                channel_multiplier=-1)
                        if i + 2 < ni:
                            pss.append(scores_mm(i + 2))
                        nc.tensor.matmul(acc_o[:, d0:],
                                         v_sb[:, 128 * i:128 * (i + 1)],
                                         E[:, d0:], start=(i == 0),
                                         stop=(i == ni - 1))
                        with nc.allow_low_precision(reason="softmax denom"):
                            if i == 0:
                                nc.vector.tensor_copy(dsA[:], E[:])
                            else:
                                nc.vector.tensor_add(dsA[:, d0:], dsA[:, d0:],
                                                     E[:, d0:])
                    nc.tensor.matmul(acc_d[:], ones[:, 0:1], dsA[:],
                                     start=True, stop=True)
                    recip = rcp.tile([1, 512], f32, tag="recip")
                    nc.vector.reciprocal_approx_fast(recip[:], acc_d[:])
                    recip_bf = rcp.tile([1, 512], bf16, tag="recipb")
                    with nc.allow_low_precision(reason="softmax denom"):
                        nc.vector.tensor_copy(recip_bf[:], recip[:])
                    bc = smp.tile([128, 512], bf16, tag="bc")
                    nc.gpsimd.partition_broadcast(bc[:], recip_bf[:])
                    ao = aop.tile([128, 512], bf16, tag="ao")
                    nc.vector.tensor_mul(ao[:], acc_o[:], bc[:])
                    nc.scalar.dma_start(
                        cc_in[j][128 * h:128 * (h + 1), :], ao[:])

                    # last chunk: fire the first half-gather once heads 0-1
                    # are out, so its latency hides behind heads 2-3
                    if with_collective and j == NCHUNK - 1 and h == 1:
                        nc.gpsimd.collective_compute(
                            "AllGather", mybir.AluOpType.bypass,
                            replica_groups=[list(range(n_cores))],
                            ins=[cc_in[j][0:256, :].opt()],
                            outs=[cc_out3[0][:].opt()])

                # ---------------- AllGather for chunk j
                if with_collective:
                    if j < NCHUNK - 1:
                        nc.gpsimd.collective_compute(
                            "AllGather", mybir.AluOpType.bypass,
                            replica_groups=[list(range(n_cores))],
                            ins=[cc_in[j][:].opt()], outs=[cc_out[j][:].opt()])
                    else:
                        nc.gpsimd.collective_compute(
                            "AllGather", mybir.AluOpType.bypass,
                            replica_groups=[list(range(n_cores))],
                            ins=[cc_in[j][256:512, :].opt()],
                            outs=[cc_out3[1][:].opt()])

                if j < NCHUNK - 1:
                    # next chunk's hidden loads BEFORE the aT issues that
                    # wait on collectives (sync queue is in-order)
                    hT_next = load_hT(j + 1)
                else:
                    # both staging loads after the last trigger: with bufs=1
                    # tiles they carry WAR deps on O2's reads, so anything
                    # emitted behind them would deadlock A3
                    o_tiles["3a"] = phase_o_load_a(NCHUNK - 1)
                    o_tiles["3b"] = phase_o_load_b(NCHUNK - 1)

                # ---------------- phase O for chunk j-1 (collective hidden
                # behind chunk j's projections+attention)
                if j >= 1:
                    phase_o_mm(j - 1, *o_tiles.pop(j - 1))
            phase_o_mm(NCHUNK - 1, o_tiles.pop("3a"), o_tiles.pop("3b"))

    nc.compile()
    return nc


# ---------------------------------------------------------------- run machinery
class _Runner:
    """Persistent PJRT runner (caches the jitted executable)."""

    def __init__(self, nc, n_cores):
        import jax
        from jax.experimental.shard_map import shard_map
        from jax.sharding import Mesh, PartitionSpec
        from concourse import bass2jax, mybir as mb

        bass2jax.install_neuronx_cc_hook()
        self.jax = jax
        self.n = n_cores
        part_name = (nc.partition_id_tensor.name
                     if nc.partition_id_tensor else None)
        in_names, out_names, out_avals, zero_shapes = [], [], [], []
        for alloc in nc.m.functions[0].allocations:
            if not isinstance(alloc, mb.MemoryLocationSet):
                continue
            name = alloc.memorylocations[0].name
            if alloc.kind == "ExternalInput":
                if name == part_name:
                    continue
                in_names.append(name)
            elif alloc.kind == "ExternalOutput":
                out_names.append(name)
                shape = tuple(alloc.tensor_shape)
                dtype = mb.dt.np(alloc.dtype)
                out_avals.append(jax.core.ShapedArray(shape, dtype))
                zero_shapes.append((shape, dtype))
        self.in_names, self.out_names = in_names, out_names
        self.out_avals, self.zero_shapes = out_avals, zero_shapes
        n_params = len(in_names)
        all_names = tuple(in_names + out_names
                          + ([part_name] if part_name else []))
        donate = tuple(range(n_params, n_params + len(out_names)))

        def _body(*args):
            operands = list(args)
            if part_name is not None:
                operands.append(bass2jax.partition_id_tensor())
            outs = bass2jax._bass_exec_p.bind(
                *operands, out_avals=tuple(out_avals), in_names=all_names,
                out_names=tuple(out_names),
                lowering_input_output_aliases=(),
                sim_require_finite=True, sim_require_nnan=True, nc=nc)
            return tuple(outs)

        devices = jax.devices()[:n_cores]
        self.mesh = Mesh(np.asarray(devices), ("core",))
        in_specs = (PartitionSpec("core"),) * (n_params + len(out_names))
        out_specs = (PartitionSpec("core"),) * len(out_names)
        self.fn = jax.jit(
            shard_map(_body, mesh=self.mesh, in_specs=in_specs,
                      out_specs=out_specs, check_rep=False),
            donate_argnums=donate, keep_unused=True)

    def concat_inputs(self, in_maps):
        return [np.concatenate([np.asarray(m[name]) for m in in_maps], axis=0)
                for name in self.in_names]

    def zeros(self):
        return [np.zeros((self.n * s[0], *s[1:]), d)
                for (s, d) in self.zero_shapes]

    def run(self, in_maps):
        out_arrs = self.fn(*self.concat_inputs(in_maps), *self.zeros())
        return [
            {name: np.asarray(out_arrs[i]).reshape(
                self.n, *self.out_avals[i].shape)[c]
             for i, name in enumerate(self.out_names)}
            for c in range(self.n)
        ]


_STATE = {}


def _get_runner():
    if "runner" not in _STATE:
        nc = build_nc(N_CORES, with_collective=True)
        _STATE["runner"] = _Runner(nc, N_CORES)
    return _STATE["runner"]


def make_in_maps(hidden, wq, wk, wv, wo):
    consts = host_constants()
    b16 = ml_dtypes.bfloat16
    hid2d = np.asarray(hidden, dtype=np.float32).reshape(S, HID)
    # hidH[p, j, t, s] = hidden[512j+s, 128t+p]
    hidH = np.ascontiguousarray(
        hid2d.reshape(NCHUNK, 512, NT, 128).transpose(3, 0, 2, 1)
    ).astype(b16)
    wq = np.asarray(wq, dtype=np.float32)
    wk = np.asarray(wk, dtype=np.float32)
    wv = np.asarray(wv, dtype=np.float32).astype(b16)
    wo = np.asarray(wo, dtype=np.float32).astype(b16)
    # de-interleave head dim of wq/wk (per head) to match the DVE RoPE
    wqp = wq.reshape(HID, NH, HD)[:, :, PERM].reshape(HID, NH * HD).astype(b16)
    wkp = wk.reshape(HID, NKV, HD)[:, :, PERM].reshape(HID, NKV * HD
                                                       ).astype(b16)
    in_maps = []
    for c in range(N_CORES):
        wq_c = wqp[:, DQ * c:DQ * (c + 1)]          # [HID, 512]
        # wqT[p, m, t, d] = wq_c[128t+p, 128m+d]
        wqT = np.ascontiguousarray(
            wq_c.reshape(NT, 128, HPC, 128).transpose(1, 2, 0, 3))
        in_maps.append({
            "hidH": hidH,
            "wqT": wqT,
            "wkT": _tile_w(wkp[:, HD * c:HD * (c + 1)]),
            "wvT": _tile_w(wv[:, HD * c:HD * (c + 1)]),
            "woT": _tile_w(wo[:, DQ * c:DQ * (c + 1)]),
            "cosP": consts["cosP"], "sinS": consts["sinS"],
            "ident": consts["ident"], "ones": consts["ones"],
        })
    return in_maps


def kernel(hidden_states, attention_mask, wq, wk, wv, wo):
    """Full-input entry point: returns [1, S, HID] float32."""
    del attention_mask  # causal mask (-1e9 upper triangle) is hardcoded
    runner = _get_runner()
    in_maps = make_in_maps(hidden_states, wq, wk, wv, wo)
    results = runner.run(in_maps)
    out = np.concatenate([results[c]["out"] for c in range(N_CORES)], axis=1)
    return out.reshape(1, S, HID).astype(np.float32)
